# revision 1
# baseline (speedup 1.0000x reference)
"""GRAFTNET kernel for 8 Trainium2 NeuronCores.

Sharding: data-parallel over batch (B=8 -> one sample per core), per the
sharding hint. All dense linears (the FLOP bulk: rel/ent embeddings
projections, head/self/tail, e2q/e2e over [tokens, 768] concat features,
attention sim) run on device through one compiled SPMD GEMM kernel,
batch-sharded across the 8 cores. Host does index prep, the tiny LSTM
recurrence/softmax scalars and the sparse segment-sums between launches.

Self-contained: hardcodes shapes from the problem spec.
"""
import sys, os
import numpy as np

sys.path.insert(0, "/opt/trn_rl_repo")

B, E, F, Q = 8, 2000, 6000, 20
D = 256
L = 3
NUM_ENTITY, NUM_REL, NUM_WORD = 400000, 6000, 60000
PAGERANK_LAMBDA, FACT_SCALE = 0.8, 3.0
VERY_NEG = -100000000000.0
VERY_SMALL = 1e-10

NT = 6144          # padded token rows for the device GEMM (48 tiles of 128)
KMAX = 768         # padded contraction dim (6 chunks of 128)
NOUT = 256         # output features per GEMM
P = 128

_CORES = list(range(8))
_compiled = {}


def _build_gemm():
    """Device program: Y[6144, 256] = Xfm.T @ Wt  (Xfm: [768, 6144] feature-
    major, Wt: [768, 256]); fp32. One sample per core (SPMD)."""
    import concourse.bass as bass
    import concourse.bacc as bacc
    import concourse.mybir as mybir
    import concourse.tile as tile

    nc = bacc.Bacc("TRN2", target_bir_lowering=False, debug=False,
                   num_devices=len(_CORES))
    xfm = nc.dram_tensor("xfm", [KMAX, NT], mybir.dt.float32,
                         kind="ExternalInput").ap()
    wt = nc.dram_tensor("wt", [KMAX, NOUT], mybir.dt.float32,
                        kind="ExternalInput").ap()
    y = nc.dram_tensor("y", [NT, NOUT], mybir.dt.float32,
                       kind="ExternalOutput").ap()

    KC = KMAX // P     # 6 contraction chunks
    MT = NT // P       # 48 token tiles

    with tile.TileContext(nc) as tc:
        with tc.tile_pool(name="w", bufs=1) as wpool, \
             tc.tile_pool(name="x", bufs=3) as xpool, \
             tc.tile_pool(name="o", bufs=3) as opool, \
             tc.tile_pool(name="ps", bufs=2, space="PSUM") as pspool:
            wtile = wpool.tile([P, KC, NOUT], mybir.dt.float32)
            # weight resident: [128, 6, 256] <- [768, 256]
            nc.sync.dma_start(out=wtile[:],
                              in_=wt.rearrange("(c p) n -> p c n", p=P))
            for m in range(MT):
                xt = xpool.tile([P, KC, P], mybir.dt.float32)
                # [768, 128-token slice] -> [128, 6, 128]
                nc.sync.dma_start(
                    out=xt[:],
                    in_=xfm[:, m * P:(m + 1) * P].rearrange(
                        "(c p) n -> p c n", p=P))
                ps = pspool.tile([P, NOUT], mybir.dt.float32, space="PSUM")
                for k in range(KC):
                    nc.tensor.matmul(out=ps[:], lhsT=xt[:, k, :],
                                     rhs=wtile[:, k, :],
                                     start=(k == 0), stop=(k == KC - 1))
                ot = opool.tile([P, NOUT], mybir.dt.float32)
                nc.scalar.copy(out=ot[:], in_=ps[:])
                nc.sync.dma_start(out=y[m * P:(m + 1) * P, :], in_=ot[:])
    nc.compile()
    return nc


def _gemm(xfms, wts):
    """xfms: list of 8 [768, 6144] f32; wts: list of 8 [768, 256] f32.
    Returns list of 8 [6144, 256] f32 computed on the 8 NeuronCores."""
    from concourse.bass_utils import run_bass_kernel_spmd
    if "gemm" not in _compiled:
        _compiled["gemm"] = _build_gemm()
    nc = _compiled["gemm"]
    in_maps = [{"xfm": np.ascontiguousarray(x, np.float32),
                "wt": np.ascontiguousarray(w, np.float32)}
               for x, w in zip(xfms, wts)]
    res = run_bass_kernel_spmd(nc, in_maps, _CORES)
    t = getattr(res, "exec_time_ns", None)
    if t:
        _gemm.total_ns += t
    _gemm.calls += 1
    return [res.results[i]["y"] for i in range(len(_CORES))]


_gemm.total_ns = 0
_gemm.calls = 0


def _batched_linear(xs, ws, bs=None, relu=False, rows=None):
    """Per-core linear on device: y_b = x_b @ w_b.T (+ b) for 8 cores.
    xs[b]: [n_b, k_b] token-major; ws[b]: [nout<=256, k_b]. Pads to the fixed
    GEMM shape; bias/relu applied host-side (cheap elementwise)."""
    xfms, wts = [], []
    for b in range(B):
        x = xs[b]; w = ws[b]
        n, k = x.shape
        xf = np.zeros((KMAX, NT), np.float32)
        xf[:k, :n] = x.T
        wt = np.zeros((KMAX, NOUT), np.float32)
        wt[:k, :w.shape[0]] = w.T
        xfms.append(xf); wts.append(wt)
    ys = _gemm(xfms, wts)
    outs = []
    for b in range(B):
        n = xs[b].shape[0] if rows is None else rows
        y = ys[b][:n, :ws[b].shape[0]]
        if bs is not None:
            y = y + bs[b]
        if relu:
            y = np.maximum(y, 0.0)
        outs.append(y.astype(np.float32))
    return outs


def _segment_sum(rows_idx, vals, gathered, nrows):
    out = np.zeros((nrows, gathered.shape[1]), np.float32)
    np.add.at(out, rows_idx, vals[:, None] * gathered)
    return out


def kernel(**inputs):
    inp = {k: np.asarray(v) for k, v in inputs.items()}
    f32 = np.float32
    A = lambda x: np.asarray(x, dtype=f32)

    ent_tab = A(inp["entity_table"])
    rel_tab = A(inp["relation_table"])
    word_tab = A(inp["word_table"])
    le = inp["local_entity"].astype(np.int64)       # [B, E]
    kfr = inp["kb_fact_rel"].astype(np.int64)       # [B, F]
    qt = inp["query_text"].astype(np.int64)         # [B, Q]

    # ---- per-core static prep ----
    qmask = (qt != NUM_WORD).astype(f32)            # [B, Q]
    lem = (le != NUM_ENTITY).astype(f32)            # [B, E]
    e2f = inp["e2f_indices"].astype(np.int64); e2fv = A(inp["e2f_val"])
    f2e = inp["f2e_indices"].astype(np.int64); f2ev = A(inp["f2e_val"])
    edges1 = []   # per-b: (f, e, val) e2f
    edges3 = []   # per-b: (e, f, val) f2e
    for b in range(B):
        m1 = e2f[0] == b
        edges1.append((e2f[1][m1], e2f[2][m1], e2fv[m1]))
        m3 = f2e[0] == b
        edges3.append((f2e[1][m3], f2e[2][m3], f2ev[m3]))

    # ---- LSTM on host (tiny: 8x20 steps of 256-dim matvec) ----
    WihT = A(inp["lstm_Wih"]).T; WhhT = A(inp["lstm_Whh"]).T
    lb = A(inp["lstm_bih"]) + A(inp["lstm_bhh"])
    sig = lambda x: 1.0 / (1.0 + np.exp(-x))
    qh = np.zeros((B, Q, D), f32)
    for b in range(B):
        wv = word_tab[qt[b]]                         # [Q, 256]
        xwb = wv @ WihT + lb
        h = np.zeros(D, f32); c = np.zeros(D, f32)
        for t in range(Q):
            g = xwb[t] + h @ WhhT
            i_, fgate, gg, o = g[:D], g[D:2*D], g[2*D:3*D], g[3*D:]
            c = sig(fgate) * c + sig(i_) * np.tanh(gg)
            h = sig(o) * np.tanh(c)
            qh[b, t] = h
    h_last = qh[:, -1, :].copy()                     # [B, 256]

    # ---- device: local_fact_emb = rel_lin(relation_table[kfr]) [B, F, 256]
    relW = A(inp["rel_lin_W"]); relb = A(inp["rel_lin_b"])
    lfe = _batched_linear([rel_tab[kfr[b]] for b in range(B)],
                          [relW] * B, [relb] * B, rows=F)

    # ---- device: local_entity_emb = ent_lin(entity_table[le]) [B, E, 256]
    entW = A(inp["ent_lin_W"]); entb = A(inp["ent_lin_b"])
    x_e = _batched_linear([ent_tab[le[b]] for b in range(B)],
                          [entW] * B, [entb] * B, rows=E)

    # ---- device: sim raw = qh @ lfe.T  -> [B, F, 20] (as lfe @ qh.T)
    simT = _batched_linear([lfe[b] for b in range(B)],
                           [qh[b] for b in range(B)], rows=F)  # [F, 20]

    # ---- attention scalars on host ----
    W_tilde = []
    for b in range(B):
        s = simT[b].T / np.sqrt(f32(D))              # [20, F]
        s = s + ((1.0 - qmask[b]) * VERY_NEG)[:, None]
        Ex = np.exp(s - s.max(0, keepdims=True))
        soft = Ex / Ex.sum(0, keepdims=True)         # softmax over q
        Wt = (soft * (simT[b].T / np.sqrt(f32(D)))).sum(0)
        W_tilde.append(np.exp(Wt - Wt.max()).astype(f32))

    # e2f_softmax / pagerank init
    rsm, pg = [], []
    for b in range(B):
        f1, e1, v1 = edges1[b]
        sm = np.zeros(E, f32)
        np.add.at(sm, e1, v1 * W_tilde[b][f1])
        rsm.append(1.0 / np.maximum(sm, VERY_SMALL))
        pg.append(A(inp["q2e_adj_mat"])[b, :, 0].copy())

    qne = h_last.copy()                              # [B, 256]
    headW = A(inp["head_W"]); headb = A(inp["head_b"])
    selfW = A(inp["self_W"]); selfb = A(inp["self_b"])
    tailW = A(inp["tail_W"]); tailb = A(inp["tail_b"])
    q2eW = A(inp["q2e_W"]); q2eb = A(inp["q2e_b"])
    e2qW = A(inp["e2q_W"]); e2qb = A(inp["e2q_b"])
    e2eW = A(inp["e2e_W"]); e2eb = A(inp["e2e_b"])

    for i in range(L):
        # device: head(x_e) [E,256] and self(lfe) [F,256] and self(x_e)
        head_x = _batched_linear(x_e, [headW[i]] * B, [headb[i]] * B)
        self_f = _batched_linear(lfe, [selfW[i]] * B, [selfb[i]] * B)
        self_e = _batched_linear(x_e, [selfW[i]] * B, [selfb[i]] * B)

        q2e_emb = [qne[b] @ q2eW[i].T + q2eb[i] for b in range(B)]

        e2f_emb, e2f_norm = [], []
        for b in range(B):
            f1, e1, v1 = edges1[b]
            sp = _segment_sum(f1, v1, head_x[b][e1], F)
            r = pg[b] * rsm[b]
            spn = np.zeros(F, f32)
            np.add.at(spn, f1, v1 * r[e1])
            norm = W_tilde[b] * spn
            e2f_norm.append(norm)
            e2f_emb.append(np.maximum(self_f[b] + sp, 0.0) * norm[:, None])

        # device: tail(e2f_emb) [F, 256]
        tail_f = _batched_linear(e2f_emb, [tailW[i]] * B, [tailb[i]] * B)

        f2e_emb, nxt = [], []
        for b in range(B):
            e3, f3, v3 = edges3[b]
            sp3 = _segment_sum(e3, v3, tail_f[b][f3], E)
            pgn = np.zeros(E, f32)
            np.add.at(pgn, e3, v3 * e2f_norm[b][f3])
            pg[b] = PAGERANK_LAMBDA * pgn + (1.0 - PAGERANK_LAMBDA) * pg[b]
            fe = np.maximum(self_e[b] + sp3, 0.0)
            f2e_emb.append(fe)
            nxt.append(np.concatenate(
                [x_e[b], np.broadcast_to(q2e_emb[b], (E, D)),
                 FACT_SCALE * fe], axis=1))          # [E, 768]

        # device: e2q(nxt) and e2e(nxt) over [E, 768]
        e2q_out = _batched_linear(nxt, [e2qW[i]] * B, [e2qb[i]] * B)
        e2e_out = _batched_linear(nxt, [e2eW[i]] * B, [e2eb[i]] * B,
                                  relu=True)
        for b in range(B):
            qne[b] = pg[b] @ e2q_out[b]
        x_e = e2e_out

    scoreW = A(inp["score_W"]); scoreb = A(inp["score_b"])
    score = np.stack([x_e[b] @ scoreW[0] + scoreb[0] for b in range(B)])

    answer = A(inp["answer_dist"])
    loss = np.mean(np.maximum(score, 0.0) - score * answer
                   + np.log1p(np.exp(-np.abs(score))), dtype=f32)
    score_m = score + (1.0 - lem) * f32(VERY_NEG)
    pred = np.argmax(score_m, axis=1).astype(np.int32)
    pred_dist = (1.0 / (1.0 + np.exp(-score_m))).astype(f32) * lem
    if _gemm.calls:
        print(f"[kernel] device GEMM launches: {_gemm.calls}, "
              f"summed HW exec: {_gemm.total_ns} ns")
    return np.float32(loss), pred, pred_dist
